# revision 18
# baseline (speedup 1.0000x reference)
"""Multi-head attention (B=2, S=2048, D=1024, H=16) on 8 TRN2 NeuronCores.

Sharding: data-parallel over batch (2 groups of 4 cores) x head-parallel
(4 heads per core). W_q/W_k/W_v are column-sharded by head, W_o is
row-sharded; the 4 partial W_o outputs per batch are summed on the host
(the unshard step), which also undoes the device-side transposed layout.

Per-core kernel v2 (cost-model-guided rewrite of the v1 pipeline):
  - All x inputs (bf16, host-pretransposed) are SBUF-resident; DMAs are
    issued up front with the first q/k column split fine-grained so the
    projection m-loop starts as early as possible.
  - Attention runs on 256-wide sq tiles (finer causal block skip: 72 of
    128 [128k x 256sq] blocks survive vs 80 at 512-wide).
  - One exp() activation per block covers all 4 heads ([128,4,256]
    PSUM -> bf16 SBUF); causal-boundary blocks are zeroed post-exp with
    gpsimd.affine_select (idle engine) instead of DVE multiplies, and are
    emitted FIRST within each sq tile so the gpsimd latency hides behind
    the other blocks' matmuls.
  - probs/v/attn/y are bf16 (DVE 2x modes + half DMA traffic); scores
    operands stay f32r.
  - v carries a ones column per head so the P@V accumulation yields the
    softmax denominator for free; the v bias is folded into bo on the
    host (attn' = attn + bv => y += Wo @ bv).
  - Normalize: one reciprocal per sq tile ([1,4,256]), rank-1 PE
    broadcast into PSUM, DVE multiply reads both PSUM operands directly.
"""

import os

import numpy as np

_B, _S, _D, _H, _DK = 2, 2048, 1024, 16, 64
_HPC = 4          # heads per core
_NCORES = 8
_CPG = 4          # cores per (batch) group
_DPC = _HPC * _DK # 256 projection dims per core
_NEG = -1e9
_SQW = 256        # sq tile width for attention
_NSQ = _S // _SQW # 8 sq tiles

_program_cache = {}
LAST_RESULTS = None  # BassKernelResults of the most recent run (for profiling)


def _analyze_mask(mask):
    """Classify each [128 k, 256 sq] block of mask^T. Returns (plan, dense).

    plan[i] = tuple of (j, mode, param) for sq-tile i; mode 0 = no mask,
    1 = causal affine_select (param = base), 2 = dense additive mask
    (param = index into dense blocks). Fully-masked blocks are omitted.
    Within a tile, masked (mode!=0) blocks come first so their extra
    post-exp work overlaps the remaining blocks' matmuls.
    """
    maskT = np.ascontiguousarray(mask.T)
    plan = []
    dense = []
    p_idx = np.arange(128)[:, None]
    s_idx = np.arange(_SQW)[None, :]
    for i in range(_NSQ):
        row = []
        for j in range(_S // 128):
            blk = maskT[j * 128:(j + 1) * 128, i * _SQW:(i + 1) * _SQW]
            nz = blk != 0.0
            if nz.all():
                continue  # fully masked: block contributes nothing
            if not nz.any():
                row.append((j, 0, 0))
                continue
            base = i * _SQW - j * 128
            causal = (s_idx + i * _SQW) < (p_idx + j * 128)
            if np.array_equal(nz, causal) and np.all(blk[nz] == 1.0):
                row.append((j, 1, base))
            else:
                row.append((j, 2, len(dense)))
                dense.append(blk * np.float32(_NEG))
        row.sort(key=lambda b: b[1] == 0)  # masked blocks first
        plan.append(tuple(row))
    if dense:
        dense_np = np.stack(dense).astype(np.float32)
    else:
        dense_np = np.zeros((1, 128, _SQW), np.float32)
    return tuple(plan), dense_np


def _build_program(plan, nblk):
    import concourse.bass as bass  # noqa: F401  (registers engine classes)
    import concourse.tile as tile
    from concourse import bacc, mybir

    F32 = mybir.dt.float32
    F32R = mybir.dt.float32r
    BF16 = mybir.dt.bfloat16
    AF = mybir.ActivationFunctionType
    ALU = mybir.AluOpType
    ts = bass.ts

    nc = bacc.Bacc(None, target_bir_lowering=False, debug=False)

    xq = nc.dram_tensor("xq", [_D, _S], BF16, kind="ExternalInput").ap()
    xk = nc.dram_tensor("xk", [_D, _S], BF16, kind="ExternalInput").ap()
    xv = nc.dram_tensor("xv", [_D, _S], BF16, kind="ExternalInput").ap()
    wq = nc.dram_tensor("wq", [_D, _DPC], BF16, kind="ExternalInput").ap()
    wk = nc.dram_tensor("wk", [_D, _DPC], BF16, kind="ExternalInput").ap()
    wv = nc.dram_tensor("wv", [_D, _DPC], BF16, kind="ExternalInput").ap()
    wo = nc.dram_tensor("wo", [_DPC, _D], F32R, kind="ExternalInput").ap()
    bq = nc.dram_tensor("bq", [_DPC], F32, kind="ExternalInput").ap()
    bk = nc.dram_tensor("bk", [_DPC], F32, kind="ExternalInput").ap()
    mblk = nc.dram_tensor("mblk", [nblk, 128, _SQW], F32,
                          kind="ExternalInput").ap()
    y = nc.dram_tensor("y", [_D, _S], BF16, kind="ExternalOutput").ap()

    has_dense = any(m == 2 for row in plan for (_, m, _) in row)
    resident_mask = has_dense and nblk <= 8
    need_stream = has_dense and not resident_mask

    with tile.TileContext(nc) as tc:
        from contextlib import ExitStack
        with ExitStack() as ctx:
            wpool = ctx.enter_context(tc.tile_pool(name="w", bufs=1))
            cpool = ctx.enter_context(tc.tile_pool(name="const", bufs=1))
            xpool = ctx.enter_context(tc.tile_pool(name="xres", bufs=1))
            biga = ctx.enter_context(tc.tile_pool(name="biga", bufs=1))
            probp = ctx.enter_context(tc.tile_pool(name="probs", bufs=8))
            recp = ctx.enter_context(tc.tile_pool(name="rec", bufs=2))
            yp = ctx.enter_context(tc.tile_pool(name="y", bufs=4))
            mpool = (
                ctx.enter_context(tc.tile_pool(name="mstream", bufs=3))
                if need_stream else None
            )
            spsp = ctx.enter_context(tc.tile_pool(name="sps", bufs=1,
                                                  space="PSUM"))
            accp = ctx.enter_context(tc.tile_pool(name="acc", bufs=4,
                                                  space="PSUM"))
            mmps = ctx.enter_context(tc.tile_pool(name="mmps", bufs=2,
                                                  space="PSUM"))

            xq_r = xq.rearrange("(m p) s -> p m s", p=128)
            xk_r = xk.rearrange("(m p) s -> p m s", p=128)
            xv_r = xv.rearrange("(m p) s -> p m s", p=128)
            wq_r = wq.rearrange("(m p) d -> p m d", p=128)
            wk_r = wk.rearrange("(m p) d -> p m d", p=128)
            wv_r = wv.rearrange("(m p) d -> p m d", p=128)

            # resident SBUF state
            xq_sb = xpool.tile([128, 8, _S], BF16, tag="xq")
            xk_sb = xpool.tile([128, 8, _S], BF16, tag="xk")
            xv_sb = xpool.tile([128, 8, _S], BF16, tag="xv")
            wq_sb = wpool.tile([128, 8, _DPC], BF16, tag="wq")
            wk_sb = wpool.tile([128, 8, _DPC], BF16, tag="wk")
            wv_sb = wpool.tile([128, 8, _DPC], BF16, tag="wv")
            wo_sb = wpool.tile([128, 2, _D], F32R, tag="wo")

            # --- DMA emission: critical path (col 0 of q/k) first,
            # fine-grained so the projection m-loops start early
            for lo, hi in ((0, 4), (4, 8)):
                nc.sync.dma_start(out=wq_sb[:, lo:hi, :], in_=wq_r[:, lo:hi, :])
            for lo in range(0, 8, 2):
                nc.sync.dma_start(out=xq_sb[:, lo:lo + 2, 0:512],
                                  in_=xq_r[:, lo:lo + 2, 0:512])
            for lo, hi in ((0, 4), (4, 8)):
                nc.sync.dma_start(out=wk_sb[:, lo:hi, :], in_=wk_r[:, lo:hi, :])
            for lo in range(0, 8, 2):
                nc.sync.dma_start(out=xk_sb[:, lo:lo + 2, 0:512],
                                  in_=xk_r[:, lo:lo + 2, 0:512])
            nc.sync.dma_start(out=wv_sb, in_=wv_r)
            for lo, hi in ((0, 4), (4, 8)):
                nc.sync.dma_start(out=xv_sb[:, lo:hi, 0:512],
                                  in_=xv_r[:, lo:hi, 0:512])
            bq_sb = cpool.tile([128, 2], F32, tag="bq")
            nc.sync.dma_start(out=bq_sb, in_=bq.rearrange("(h p) -> p h", p=128))
            bk_sb = cpool.tile([128, 2], F32, tag="bk")
            nc.sync.dma_start(out=bk_sb, in_=bk.rearrange("(h p) -> p h", p=128))
            for st in (1, 2, 3):
                nc.sync.dma_start(out=xq_sb[:, :, ts(st, 512)],
                                  in_=xq_r[:, :, ts(st, 512)])
                nc.sync.dma_start(out=xk_sb[:, :, ts(st, 512)],
                                  in_=xk_r[:, :, ts(st, 512)])
                nc.sync.dma_start(out=xv_sb[:, :, ts(st, 512)],
                                  in_=xv_r[:, :, ts(st, 512)])
            nc.sync.dma_start(out=wo_sb, in_=wo.rearrange("(c p) o -> p c o", p=128))
            if resident_mask:
                mask_sb = cpool.tile([128, nblk, _SQW], F32, tag="mask")
                nc.sync.dma_start(out=mask_sb,
                                  in_=mblk.rearrange("n p s -> p n s"))

            # --- big SBUF state ---
            qT = biga.tile([128, 2, _S], F32R, tag="qT")
            kT = biga.tile([128, 2, _S], F32R, tag="kT")
            vsb = biga.tile([128, 16, _HPC * 65], BF16, tag="v")
            attn = biga.tile([128, 2, _S], F32R, tag="attn")

            # ones columns of v (softmax denominator trick): one strided memset
            nc.vector.memset(
                vsb.rearrange("p a (h x) -> p a h x", x=65)[:, :, :, 64:65], 1.0
            )

            # v-projection emitted lazily per 512-wide k-column group
            v_pending = set(range(4))

            def ensure_vgroup(col):
                if col not in v_pending:
                    return
                v_pending.discard(col)
                for c in range(4):
                    sc = col * 4 + c
                    vps = mmps.tile([128, _DPC], F32, tag="mm", name="vps")
                    for m in range(8):
                        nc.tensor.matmul(
                            vps, lhsT=xv_sb[:, m, ts(sc, 128)],
                            rhs=wv_sb[:, m, :], start=(m == 0), stop=(m == 7),
                        )
                    nc.vector.tensor_copy(
                        vsb[:, sc, 0:260].rearrange(
                            "p (h x) -> p h x", x=65)[:, :, 0:64],
                        vps.rearrange("p (h x) -> p h x", x=64),
                    )

            def emit_proj(st):
                # q/k projections for 512-wide column st
                for dh in range(2):
                    qps = mmps.tile([128, 512], F32, tag="mm", name="qps")
                    for m in range(8):
                        nc.tensor.matmul(
                            qps, lhsT=wq_sb[:, m, ts(dh, 128)],
                            rhs=xq_sb[:, m, ts(st, 512)],
                            start=(m == 0), stop=(m == 7),
                        )
                    nc.vector.tensor_scalar(
                        qT[:, dh, ts(st, 512)], qps, bq_sb[:, dh:dh + 1], None,
                        ALU.add,
                    )
                    kps = mmps.tile([128, 512], F32, tag="mm", name="kps")
                    for m in range(8):
                        nc.tensor.matmul(
                            kps, lhsT=wk_sb[:, m, ts(dh, 128)],
                            rhs=xk_sb[:, m, ts(st, 512)],
                            start=(m == 0), stop=(m == 7),
                        )
                    nc.vector.tensor_scalar(
                        kT[:, dh, ts(st, 512)], kps, bk_sb[:, dh:dh + 1], None,
                        ALU.add,
                    )

            sps = spsp.tile([128, 4, _SQW], F32, tag="sps", name="sps")

            def emit_attn(i):
                blocks = plan[i]
                nj = len(blocks)
                if nj == 0:
                    return
                acc = [accp.tile([65, 512], F32, tag="acc", name=f"acc{i}_{h}")
                       for h in range(4)]
                for bi, (j, mode, param) in enumerate(blocks):
                    ensure_vgroup(j // 4)
                    for h in range(4):
                        # slot order hh-major: bank0 holds the two
                        # partition-0:64 heads, bank1 the partition-64:128
                        # heads -- consecutive matmuls into one psum bank
                        # must share a tile position (device constraint)
                        hh, g = h // 2, h % 2
                        nc.tensor.matmul(
                            sps[:, h, :],
                            lhsT=kT[hh * 64:(hh + 1) * 64, g, ts(j, 128)],
                            rhs=qT[hh * 64:(hh + 1) * 64, g, ts(i, _SQW)],
                            start=True, stop=True,
                        )
                    if mode == 2:
                        if resident_mask:
                            mt = mask_sb[:, param, :]
                        else:
                            mt = mpool.tile([128, _SQW], F32, tag="mtile",
                                            name="mt")
                            nc.sync.dma_start(out=mt, in_=mblk[param])
                        for h in range(4):
                            nc.vector.tensor_add(sps[:, h, :], sps[:, h, :], mt)
                    # exp in per-bank halves: block b+1's scores into bank0
                    # only wait on the bank0 half-exp of block b (subtile WAR)
                    prob2 = []
                    ncols = min(_SQW, 128 - param) if mode == 1 else 0
                    for half in range(2):
                        pr = probp.tile([128, 2, _SQW], BF16, tag="probs",
                                        name=f"probs{half}")
                        nc.scalar.activation(pr, sps[:, 2 * half:2 * half + 2, :],
                                             AF.Exp)
                        if mode == 1 and ncols > 0:
                            nc.gpsimd.affine_select(
                                out=pr[:, :, 0:ncols],
                                in_=pr[:, :, 0:ncols],
                                compare_op=ALU.is_ge, fill=0.0,
                                base=param, channel_multiplier=-1,
                                pattern=[[0, 2], [1, ncols]],
                            )
                        prob2.append(pr)
                    for h in range(4):
                        hh, g = h // 2, h % 2
                        hv = 2 * g + hh
                        nc.tensor.matmul(
                            acc[h][:, 0:_SQW],
                            lhsT=vsb[:, j, hv * 65:(hv + 1) * 65],
                            rhs=prob2[h // 2][:, h % 2, :],
                            start=(bi == 0), stop=(bi == nj - 1),
                        )
                # softmax normalize: acc row 64 holds the denominators;
                # reciprocal on DVE, partition-broadcast on idle gpsimd
                rec = recp.tile([1, 4, _SQW], F32, tag="rec", name="rec")
                with nc.allow_low_precision(
                    reason="softmax reciprocal"
                ):
                    for h in range(4):
                        nc.vector.reciprocal(rec[:, h, :],
                                             acc[h][64:65, 0:_SQW])
                recbc = recp.tile([64, 4, _SQW], F32, tag="recbc",
                                  name="recbc")
                nc.gpsimd.partition_broadcast(recbc, rec)
                for h in range(4):
                    hh, g = h // 2, h % 2
                    nc.vector.tensor_mul(
                        attn[hh * 64:(hh + 1) * 64, g, ts(i, _SQW)],
                        acc[h][0:64, 0:_SQW], recbc[:, h, :],
                    )

            def emit_outproj(st):
                # output projection for 512-wide column st (row-sharded partial)
                for oc in range(8):
                    yps = mmps.tile([128, 512], F32, tag="mm", name="yps")
                    for cc in range(2):
                        nc.tensor.matmul(
                            yps, lhsT=wo_sb[:, cc, ts(oc, 128)],
                            rhs=attn[:, cc, ts(st, 512)],
                            start=(cc == 0), stop=(cc == 1),
                        )
                    y_sb = yp.tile([128, 512], BF16, tag="y", name="y_sb")
                    nc.vector.tensor_copy(y_sb, yps)
                    nc.sync.dma_start(
                        out=y[oc * 128:(oc + 1) * 128, ts(st, 512)], in_=y_sb
                    )

            for st in range(4):
                emit_proj(st)
                emit_attn(2 * st)
                emit_attn(2 * st + 1)
                emit_outproj(st)

    nc.compile()
    return nc


def kernel(**inputs):
    global LAST_RESULTS
    from concourse.bass_utils import run_bass_kernel_spmd

    Q = np.asarray(inputs["Q"], dtype=np.float32)
    K = np.asarray(inputs["K"], dtype=np.float32)
    V = np.asarray(inputs["V"], dtype=np.float32)
    mask = np.asarray(inputs["mask"], dtype=np.float32)
    Wq = np.asarray(inputs["Wq"], dtype=np.float32)
    bq = np.asarray(inputs["bq"], dtype=np.float32)
    Wk = np.asarray(inputs["Wk"], dtype=np.float32)
    bk = np.asarray(inputs["bk"], dtype=np.float32)
    Wv = np.asarray(inputs["Wv"], dtype=np.float32)
    bv = np.asarray(inputs["bv"], dtype=np.float32)
    Wo = np.asarray(inputs["Wo"], dtype=np.float32)
    bo = np.asarray(inputs["bo"], dtype=np.float32)

    plan, dense = _analyze_mask(mask)
    key = (plan, dense.shape[0])
    if key not in _program_cache:
        _program_cache[key] = _build_program(plan, dense.shape[0])
    nc = _program_cache[key]

    import ml_dtypes
    bf16 = ml_dtypes.bfloat16
    sc = np.float32(1.0 / np.sqrt(_DK))
    xqT = [np.ascontiguousarray(Q[b].T).astype(bf16) for b in range(_B)]
    xkT = [np.ascontiguousarray(K[b].T).astype(bf16) for b in range(_B)]
    xvT = [np.ascontiguousarray(V[b].T).astype(bf16) for b in range(_B)]

    in_maps = []
    for core in range(_NCORES):
        b = core // _CPG
        rows = slice((core % _CPG) * _DPC, (core % _CPG) * _DPC + _DPC)
        in_maps.append({
            "xq": xqT[b], "xk": xkT[b], "xv": xvT[b],
            "wq": np.ascontiguousarray((Wq[rows] * sc).T).astype(bf16),
            "wk": np.ascontiguousarray(Wk[rows].T).astype(bf16),
            "wv": np.ascontiguousarray(Wv[rows].T).astype(bf16),
            "wo": np.ascontiguousarray(Wo[:, rows].T),
            "bq": np.ascontiguousarray(bq[rows] * sc),
            "bk": np.ascontiguousarray(bk[rows]),
            "mblk": dense,
        })

    trace = bool(int(os.environ.get("KERNEL_TRACE", "0")))
    LAST_RESULTS = run_bass_kernel_spmd(
        nc, in_maps, list(range(_NCORES)), trace=trace
    )

    # v-bias folded into the output bias: attn' = attn + bv per token
    bo_eff = bo.astype(np.float64) + Wo.astype(np.float64) @ bv.astype(np.float64)
    out = np.empty((_B, _S, _D), np.float32)
    for b in range(_B):
        acc = np.zeros((_D, _S), np.float64)
        for c in range(_CPG):
            acc += np.asarray(LAST_RESULTS.results[b * _CPG + c]["y"],
                              dtype=np.float64)
        out[b] = (acc.T + bo_eff).astype(np.float32)
    return out


# revision 19
# speedup vs baseline: 1.1889x; 1.1889x over previous
"""Multi-head attention (B=2, S=2048, D=1024, H=16) on 8 TRN2 NeuronCores.

Sharding: data-parallel over batch (2 groups of 4 cores) x head-parallel
(4 heads per core). W_q/W_k/W_v are column-sharded by head, W_o is
row-sharded; the 4 partial W_o outputs per batch are summed on the host
(the unshard step), which also undoes the device-side transposed layout.

Per-core kernel v2 (cost-model-guided rewrite of the v1 pipeline):
  - All x inputs (bf16, host-pretransposed) are SBUF-resident; DMAs are
    issued up front with the first q/k column split fine-grained so the
    projection m-loop starts as early as possible.
  - Attention runs on 256-wide sq tiles (finer causal block skip: 72 of
    128 [128k x 256sq] blocks survive vs 80 at 512-wide).
  - One exp() activation per block covers all 4 heads ([128,4,256]
    PSUM -> bf16 SBUF); causal-boundary blocks are zeroed post-exp with
    gpsimd.affine_select (idle engine) instead of DVE multiplies, and are
    emitted FIRST within each sq tile so the gpsimd latency hides behind
    the other blocks' matmuls.
  - probs/v/attn/y are bf16 (DVE 2x modes + half DMA traffic); scores
    operands stay f32r.
  - v carries a ones column per head so the P@V accumulation yields the
    softmax denominator for free; the v bias is folded into bo on the
    host (attn' = attn + bv => y += Wo @ bv).
  - Normalize: one reciprocal per sq tile ([1,4,256]), rank-1 PE
    broadcast into PSUM, DVE multiply reads both PSUM operands directly.
"""

import os

import numpy as np

_B, _S, _D, _H, _DK = 2, 2048, 1024, 16, 64
_HPC = 4          # heads per core
_NCORES = 8
_CPG = 4          # cores per (batch) group
_DPC = _HPC * _DK # 256 projection dims per core
_NEG = -1e9
_SQW = 256        # sq tile width for attention
_NSQ = _S // _SQW # 8 sq tiles

_program_cache = {}
LAST_RESULTS = None  # BassKernelResults of the most recent run (for profiling)


def _analyze_mask(mask):
    """Classify each [128 k, 256 sq] block of mask^T. Returns (plan, dense).

    plan[i] = tuple of (j, mode, param) for sq-tile i; mode 0 = no mask,
    1 = causal affine_select (param = base), 2 = dense additive mask
    (param = index into dense blocks). Fully-masked blocks are omitted.
    Within a tile, masked (mode!=0) blocks come first so their extra
    post-exp work overlaps the remaining blocks' matmuls.
    """
    maskT = np.ascontiguousarray(mask.T)
    plan = []
    dense = []
    p_idx = np.arange(128)[:, None]
    s_idx = np.arange(_SQW)[None, :]
    for i in range(_NSQ):
        row = []
        for j in range(_S // 128):
            blk = maskT[j * 128:(j + 1) * 128, i * _SQW:(i + 1) * _SQW]
            nz = blk != 0.0
            if nz.all():
                continue  # fully masked: block contributes nothing
            if not nz.any():
                row.append((j, 0, 0))
                continue
            base = i * _SQW - j * 128
            causal = (s_idx + i * _SQW) < (p_idx + j * 128)
            if np.array_equal(nz, causal) and np.all(blk[nz] == 1.0):
                row.append((j, 1, base))
            else:
                row.append((j, 2, len(dense)))
                dense.append(blk * np.float32(_NEG))
        row.sort(key=lambda b: b[1] == 0)  # masked blocks first
        plan.append(tuple(row))
    if dense:
        dense_np = np.stack(dense).astype(np.float32)
    else:
        dense_np = np.zeros((1, 128, _SQW), np.float32)
    return tuple(plan), dense_np


def _build_program(plan, nblk):
    import concourse.bass as bass  # noqa: F401  (registers engine classes)
    import concourse.tile as tile
    from concourse import bacc, mybir

    F32 = mybir.dt.float32
    F32R = mybir.dt.float32r
    BF16 = mybir.dt.bfloat16
    AF = mybir.ActivationFunctionType
    ALU = mybir.AluOpType
    ts = bass.ts

    nc = bacc.Bacc(None, target_bir_lowering=False, debug=False)

    xq = nc.dram_tensor("xq", [_D, _S], BF16, kind="ExternalInput").ap()
    xk = nc.dram_tensor("xk", [_D, _S], BF16, kind="ExternalInput").ap()
    xv = nc.dram_tensor("xv", [_D, _S], BF16, kind="ExternalInput").ap()
    wq = nc.dram_tensor("wq", [_D, _DPC], BF16, kind="ExternalInput").ap()
    wk = nc.dram_tensor("wk", [_D, _DPC], BF16, kind="ExternalInput").ap()
    wv = nc.dram_tensor("wv", [_D, _DPC], BF16, kind="ExternalInput").ap()
    wo = nc.dram_tensor("wo", [_DPC, _D], F32R, kind="ExternalInput").ap()
    bq = nc.dram_tensor("bq", [_DPC], F32, kind="ExternalInput").ap()
    bk = nc.dram_tensor("bk", [_DPC], F32, kind="ExternalInput").ap()
    mblk = nc.dram_tensor("mblk", [nblk, 128, _SQW], F32,
                          kind="ExternalInput").ap()
    y = nc.dram_tensor("y", [_D, _S], BF16, kind="ExternalOutput").ap()

    has_dense = any(m == 2 for row in plan for (_, m, _) in row)
    resident_mask = has_dense and nblk <= 8
    need_stream = has_dense and not resident_mask

    with tile.TileContext(nc) as tc:
        from contextlib import ExitStack
        with ExitStack() as ctx:
            wpool = ctx.enter_context(tc.tile_pool(name="w", bufs=1))
            cpool = ctx.enter_context(tc.tile_pool(name="const", bufs=1))
            xpool = ctx.enter_context(tc.tile_pool(name="xres", bufs=1))
            biga = ctx.enter_context(tc.tile_pool(name="biga", bufs=1))
            probp = ctx.enter_context(tc.tile_pool(name="probs", bufs=8))
            recp = ctx.enter_context(tc.tile_pool(name="rec", bufs=2))
            yp = ctx.enter_context(tc.tile_pool(name="y", bufs=4))
            mpool = (
                ctx.enter_context(tc.tile_pool(name="mstream", bufs=3))
                if need_stream else None
            )
            spsp = ctx.enter_context(tc.tile_pool(name="sps", bufs=1,
                                                  space="PSUM"))
            accp = ctx.enter_context(tc.tile_pool(name="acc", bufs=4,
                                                  space="PSUM"))
            mmps = ctx.enter_context(tc.tile_pool(name="mmps", bufs=1,
                                                  space="PSUM"))
            omps = ctx.enter_context(tc.tile_pool(name="omps", bufs=1,
                                                  space="PSUM"))

            xq_r = xq.rearrange("(m p) s -> p m s", p=128)
            xk_r = xk.rearrange("(m p) s -> p m s", p=128)
            xv_r = xv.rearrange("(m p) s -> p m s", p=128)
            wq_r = wq.rearrange("(m p) d -> p m d", p=128)
            wk_r = wk.rearrange("(m p) d -> p m d", p=128)
            wv_r = wv.rearrange("(m p) d -> p m d", p=128)

            # resident SBUF state
            xq_sb = xpool.tile([128, 8, _S], BF16, tag="xq")
            xk_sb = xpool.tile([128, 8, _S], BF16, tag="xk")
            xv_sb = xpool.tile([128, 8, _S], BF16, tag="xv")
            wq_sb = wpool.tile([128, 8, _DPC], BF16, tag="wq")
            wk_sb = wpool.tile([128, 8, _DPC], BF16, tag="wk")
            wv_sb = wpool.tile([128, 8, _DPC], BF16, tag="wv")
            wo_sb = wpool.tile([128, 2, _D], F32R, tag="wo")

            # --- DMA emission: critical path (col 0 of q/k) first,
            # fine-grained so the projection m-loops start early
            for lo, hi in ((0, 4), (4, 8)):
                nc.sync.dma_start(out=wq_sb[:, lo:hi, :], in_=wq_r[:, lo:hi, :])
            for lo in range(0, 8, 2):
                nc.sync.dma_start(out=xq_sb[:, lo:lo + 2, 0:512],
                                  in_=xq_r[:, lo:lo + 2, 0:512])
            for lo, hi in ((0, 4), (4, 8)):
                nc.sync.dma_start(out=wk_sb[:, lo:hi, :], in_=wk_r[:, lo:hi, :])
            for lo in range(0, 8, 2):
                nc.sync.dma_start(out=xk_sb[:, lo:lo + 2, 0:512],
                                  in_=xk_r[:, lo:lo + 2, 0:512])
            nc.sync.dma_start(out=wv_sb, in_=wv_r)
            for lo, hi in ((0, 4), (4, 8)):
                nc.sync.dma_start(out=xv_sb[:, lo:hi, 0:512],
                                  in_=xv_r[:, lo:hi, 0:512])
            bq_sb = cpool.tile([128, 2], F32, tag="bq")
            nc.sync.dma_start(out=bq_sb, in_=bq.rearrange("(h p) -> p h", p=128))
            bk_sb = cpool.tile([128, 2], F32, tag="bk")
            nc.sync.dma_start(out=bk_sb, in_=bk.rearrange("(h p) -> p h", p=128))
            for st in (1, 2, 3):
                nc.sync.dma_start(out=xq_sb[:, :, ts(st, 512)],
                                  in_=xq_r[:, :, ts(st, 512)])
                nc.sync.dma_start(out=xk_sb[:, :, ts(st, 512)],
                                  in_=xk_r[:, :, ts(st, 512)])
                nc.sync.dma_start(out=xv_sb[:, :, ts(st, 512)],
                                  in_=xv_r[:, :, ts(st, 512)])
            nc.sync.dma_start(out=wo_sb, in_=wo.rearrange("(c p) o -> p c o", p=128))
            if resident_mask:
                mask_sb = cpool.tile([128, nblk, _SQW], F32, tag="mask")
                nc.sync.dma_start(out=mask_sb,
                                  in_=mblk.rearrange("n p s -> p n s"))

            # --- big SBUF state ---
            qT = biga.tile([128, 2, _S], F32R, tag="qT")
            kT = biga.tile([128, 2, _S], F32R, tag="kT")
            vsb = biga.tile([128, 16, _HPC * 65], BF16, tag="v")
            attn = biga.tile([128, 2, _S], F32R, tag="attn")

            # ones columns of v (softmax denominator trick): one strided memset
            nc.vector.memset(
                vsb.rearrange("p a (h x) -> p a h x", x=65)[:, :, :, 64:65], 1.0
            )

            # v-projection emitted lazily per 512-wide k-column group
            v_pending = set(range(4))

            def ensure_vgroup(col):
                if col not in v_pending:
                    return
                v_pending.discard(col)
                for c in range(4):
                    sc = col * 4 + c
                    vps = mmps.tile([128, _DPC], F32, tag="mm", name="vps")
                    for m in range(8):
                        nc.tensor.matmul(
                            vps, lhsT=xv_sb[:, m, ts(sc, 128)],
                            rhs=wv_sb[:, m, :], start=(m == 0), stop=(m == 7),
                        )
                    nc.vector.tensor_copy(
                        vsb[:, sc, 0:260].rearrange(
                            "p (h x) -> p h x", x=65)[:, :, 0:64],
                        vps.rearrange("p (h x) -> p h x", x=64),
                    )

            def emit_proj(st):
                # q/k projections for 512-wide column st
                for dh in range(2):
                    qps = mmps.tile([128, 512], F32, tag="mm", name="qps")
                    for m in range(8):
                        nc.tensor.matmul(
                            qps, lhsT=wq_sb[:, m, ts(dh, 128)],
                            rhs=xq_sb[:, m, ts(st, 512)],
                            start=(m == 0), stop=(m == 7),
                        )
                    nc.vector.tensor_scalar(
                        qT[:, dh, ts(st, 512)], qps, bq_sb[:, dh:dh + 1], None,
                        ALU.add,
                    )
                    kps = mmps.tile([128, 512], F32, tag="mm", name="kps")
                    for m in range(8):
                        nc.tensor.matmul(
                            kps, lhsT=wk_sb[:, m, ts(dh, 128)],
                            rhs=xk_sb[:, m, ts(st, 512)],
                            start=(m == 0), stop=(m == 7),
                        )
                    nc.vector.tensor_scalar(
                        kT[:, dh, ts(st, 512)], kps, bk_sb[:, dh:dh + 1], None,
                        ALU.add,
                    )

            sps = spsp.tile([128, 4, _SQW], F32, tag="sps", name="sps")

            def emit_attn(i):
                blocks = plan[i]
                nj = len(blocks)
                if nj == 0:
                    return
                acc = [accp.tile([65, 512], F32, tag="acc", name=f"acc{i}_{h}")
                       for h in range(4)]
                for bi, (j, mode, param) in enumerate(blocks):
                    ensure_vgroup(j // 4)
                    for h in range(4):
                        # slot order hh-major: bank0 holds the two
                        # partition-0:64 heads, bank1 the partition-64:128
                        # heads -- consecutive matmuls into one psum bank
                        # must share a tile position (device constraint)
                        hh, g = h // 2, h % 2
                        nc.tensor.matmul(
                            sps[:, h, :],
                            lhsT=kT[hh * 64:(hh + 1) * 64, g, ts(j, 128)],
                            rhs=qT[hh * 64:(hh + 1) * 64, g, ts(i, _SQW)],
                            start=True, stop=True,
                        )
                    if mode == 2:
                        if resident_mask:
                            mt = mask_sb[:, param, :]
                        else:
                            mt = mpool.tile([128, _SQW], F32, tag="mtile",
                                            name="mt")
                            nc.sync.dma_start(out=mt, in_=mblk[param])
                        for h in range(4):
                            nc.vector.tensor_add(sps[:, h, :], sps[:, h, :], mt)
                    # exp in per-bank halves: block b+1's scores into bank0
                    # only wait on the bank0 half-exp of block b (subtile WAR)
                    prob2 = []
                    ncols = min(_SQW, 128 - param) if mode == 1 else 0
                    for half in range(2):
                        pr = probp.tile([128, 2, _SQW], BF16, tag="probs",
                                        name=f"probs{half}")
                        nc.scalar.activation(pr, sps[:, 2 * half:2 * half + 2, :],
                                             AF.Exp)
                        if mode == 1 and ncols > 0:
                            nc.gpsimd.affine_select(
                                out=pr[:, :, 0:ncols],
                                in_=pr[:, :, 0:ncols],
                                compare_op=ALU.is_ge, fill=0.0,
                                base=param, channel_multiplier=-1,
                                pattern=[[0, 2], [1, ncols]],
                            )
                        prob2.append(pr)
                    for h in range(4):
                        hh, g = h // 2, h % 2
                        hv = 2 * g + hh
                        nc.tensor.matmul(
                            acc[h][:, 0:_SQW],
                            lhsT=vsb[:, j, hv * 65:(hv + 1) * 65],
                            rhs=prob2[h // 2][:, h % 2, :],
                            start=(bi == 0), stop=(bi == nj - 1),
                        )
                # softmax normalize: acc row 64 holds the denominators;
                # reciprocal on DVE, partition-broadcast on idle gpsimd
                rec = recp.tile([1, 4, _SQW], F32, tag="rec", name="rec")
                with nc.allow_low_precision(
                    reason="softmax reciprocal"
                ):
                    for h in range(4):
                        nc.vector.reciprocal(rec[:, h, :],
                                             acc[h][64:65, 0:_SQW])
                recbc = recp.tile([64, 4, _SQW], F32, tag="recbc",
                                  name="recbc")
                nc.gpsimd.partition_broadcast(recbc, rec)
                for h in range(4):
                    hh, g = h // 2, h % 2
                    nc.vector.tensor_mul(
                        attn[hh * 64:(hh + 1) * 64, g, ts(i, _SQW)],
                        acc[h][0:64, 0:_SQW], recbc[:, h, :],
                    )

            def emit_outproj(st):
                # output projection for 512-wide column st (row-sharded partial)
                for oc in range(8):
                    yps = omps.tile([128, 512], F32, tag="yps", name="yps")
                    for cc in range(2):
                        nc.tensor.matmul(
                            yps, lhsT=wo_sb[:, cc, ts(oc, 128)],
                            rhs=attn[:, cc, ts(st, 512)],
                            start=(cc == 0), stop=(cc == 1),
                        )
                    y_sb = yp.tile([128, 512], BF16, tag="y", name="y_sb")
                    nc.vector.tensor_copy(y_sb, yps)
                    nc.sync.dma_start(
                        out=y[oc * 128:(oc + 1) * 128, ts(st, 512)], in_=y_sb
                    )

            for st in range(4):
                emit_proj(st)
                emit_attn(2 * st)
                emit_attn(2 * st + 1)
                emit_outproj(st)

    nc.compile()
    return nc


def kernel(**inputs):
    global LAST_RESULTS
    from concourse.bass_utils import run_bass_kernel_spmd

    Q = np.asarray(inputs["Q"], dtype=np.float32)
    K = np.asarray(inputs["K"], dtype=np.float32)
    V = np.asarray(inputs["V"], dtype=np.float32)
    mask = np.asarray(inputs["mask"], dtype=np.float32)
    Wq = np.asarray(inputs["Wq"], dtype=np.float32)
    bq = np.asarray(inputs["bq"], dtype=np.float32)
    Wk = np.asarray(inputs["Wk"], dtype=np.float32)
    bk = np.asarray(inputs["bk"], dtype=np.float32)
    Wv = np.asarray(inputs["Wv"], dtype=np.float32)
    bv = np.asarray(inputs["bv"], dtype=np.float32)
    Wo = np.asarray(inputs["Wo"], dtype=np.float32)
    bo = np.asarray(inputs["bo"], dtype=np.float32)

    plan, dense = _analyze_mask(mask)
    key = (plan, dense.shape[0])
    if key not in _program_cache:
        _program_cache[key] = _build_program(plan, dense.shape[0])
    nc = _program_cache[key]

    import ml_dtypes
    bf16 = ml_dtypes.bfloat16
    sc = np.float32(1.0 / np.sqrt(_DK))
    xqT = [np.ascontiguousarray(Q[b].T).astype(bf16) for b in range(_B)]
    xkT = [np.ascontiguousarray(K[b].T).astype(bf16) for b in range(_B)]
    xvT = [np.ascontiguousarray(V[b].T).astype(bf16) for b in range(_B)]

    in_maps = []
    for core in range(_NCORES):
        b = core // _CPG
        rows = slice((core % _CPG) * _DPC, (core % _CPG) * _DPC + _DPC)
        in_maps.append({
            "xq": xqT[b], "xk": xkT[b], "xv": xvT[b],
            "wq": np.ascontiguousarray((Wq[rows] * sc).T).astype(bf16),
            "wk": np.ascontiguousarray(Wk[rows].T).astype(bf16),
            "wv": np.ascontiguousarray(Wv[rows].T).astype(bf16),
            "wo": np.ascontiguousarray(Wo[:, rows].T),
            "bq": np.ascontiguousarray(bq[rows] * sc),
            "bk": np.ascontiguousarray(bk[rows]),
            "mblk": dense,
        })

    trace = bool(int(os.environ.get("KERNEL_TRACE", "0")))
    LAST_RESULTS = run_bass_kernel_spmd(
        nc, in_maps, list(range(_NCORES)), trace=trace
    )

    # v-bias folded into the output bias: attn' = attn + bv per token
    bo_eff = bo.astype(np.float64) + Wo.astype(np.float64) @ bv.astype(np.float64)
    out = np.empty((_B, _S, _D), np.float32)
    for b in range(_B):
        acc = np.zeros((_D, _S), np.float64)
        for c in range(_CPG):
            acc += np.asarray(LAST_RESULTS.results[b * _CPG + c]["y"],
                              dtype=np.float64)
        out[b] = (acc.T + bo_eff).astype(np.float32)
    return out
